# revision 8
# baseline (speedup 1.0000x reference)
"""Trainium2 Bass kernel for CustomPointScatter (nn_CustomPointScatter).

Reference computation:
    pillar_feat = point_features.mean(axis=1)            # [40000, 64]
    out = zeros([4, 64, 512, 512]); out[b, :, y, x] = pillar_feat

Strategy (data parallel over pillars, 8 cores):
  - Host casts point_features to fp16 (tolerance is 2e-2; fp16 error on a
    mean-of-32 is ~1e-3 relative) and splits each 2048-elem pillar row into
    point-halves lo = points 0..15, hi = points 16..31, stored as two
    [40000, 1024] arrays.  Core r gets the contiguous zero-copy slice
    [r*5000, (r+1)*5000) of each.
  - Per tile (4 pillar rows per partition), the lo half is DMA-loaded on
    the SP hardware-DGE ring and the hi half is then DMA-*accumulated* onto
    it with the SDMA engines' inline CCE adder (SWDGE accum_op=add; Tile's
    same-tile writer serialization orders it after the lo load).  That does
    the first halving-add of the reduction inside the DMA datapath for
    free, so DVE only runs the remaining 4 stages (~26 us instead of ~50).
  - DVE halving adds w=512,256,128 in place, final add writes the compact
    fp16 feature tile, which returns to DRAM on the ACT hardware-DGE ring:
    per-core [5000, 64] means.
  - 5000 rows = nine 512-row tiles + four 128-row tail tiles (the last
    overlapping by 120 rows) so the pipeline drains through small tiles.
  - Host unshard: upcast to f32, apply the exact 1/32 scale, place rows
    into the dense [4, 64, 512, 512] output at (b, :, y, x).

Per-core hardware profile: the 16 SDMA engines each carry 1/16 of the
20.5 MB load at ~26 GB/s line rate (~48 us span, the pacer), DVE ~26 us
fully hidden; prologue ~8 us is framework-fixed.
"""

import numpy as np

import concourse.bacc as bacc
import concourse.mybir as mybir
import concourse.tile as tile
from concourse.bass_utils import run_bass_kernel_spmd

B, H, W = 4, 512, 512
N_PILLARS, N_POINTS, C = 40000, 32, 64
N_CORES = 8
P = 128                  # SBUF partitions
D = N_POINTS * C         # 2048 fp16 elems per pillar row
HD = D // 2              # 1024: elems per half row
NPC = N_PILLARS // N_CORES   # 5000 pillars per core
IPB = 4                  # pillar rows per partition per (full) tile
BUFS = 10


def tile_plan(npc=NPC, ipb=IPB):
    """(start, ipb) per tile: full tiles then ipb=1 tail tiles; the final
    tail tile is shifted back so every row < npc is covered exactly."""
    full = P * ipb
    plan = []
    pos = 0
    while pos + full <= npc:
        plan.append((pos, ipb))
        pos += full
    while pos + P <= npc:
        plan.append((pos, 1))
        pos += P
    if pos < npc:
        plan.append((npc - P, 1))
    return plan


def build_nc(npc=NPC, ipb=IPB, bufs=BUFS, accum=True):
    plan = tile_plan(npc, ipb)
    nc = bacc.Bacc("TRN2", target_bir_lowering=False)
    pf_lo = nc.dram_tensor("pf_lo", [npc, HD], mybir.dt.float16,
                           kind="ExternalInput")
    pf_hi = nc.dram_tensor("pf_hi", [npc, HD], mybir.dt.float16,
                           kind="ExternalInput")
    po = nc.dram_tensor("po", [npc, C], mybir.dt.float16, kind="ExternalOutput")
    with tile.TileContext(nc) as tc:
        with (
            tc.tile_pool(name="io", bufs=bufs) as io_pool,
            tc.tile_pool(name="fo", bufs=4) as fo_pool,
        ):
            for start, tipb in plan:
                rows = slice(start, start + P * tipb)
                sb = io_pool.tile([P, tipb * HD], mybir.dt.float16, tag="sb")
                v = sb[:].rearrange("p (i w) -> p i w", w=HD)
                # pillar j = start + p*tipb + i -> partition p, block i:
                # tipb*2 KB contiguous per partition on both sides.
                nc.sync.dma_start(
                    out=sb[:],
                    in_=pf_lo[rows, :].rearrange("(p i) w -> p (i w)", p=P),
                )
                # hi half accumulates onto the lo half inside the SDMA
                # datapath (CCE add).  Same-tile WAW -> ordered after the
                # lo load by the Tile scheduler.  The SWDGE accum path only
                # accepts 2D APs with <=2048-elem descriptors (CCE element
                # limit), so issue one [P, 2048] accum per pillar-row pair.
                hflat = pf_hi[rows, :].rearrange("(p i) w -> p (i w)", p=P)
                seg = 2048
                for h0 in range(0, tipb * HD, seg):
                    h1 = min(h0 + seg, tipb * HD)
                    nc.gpsimd.dma_start(
                        out=sb[:, h0:h1],
                        in_=hflat[:, h0:h1],
                        accum_op=mybir.AluOpType.add,
                    )
                w = HD // 2
                while w > C:
                    nc.vector.tensor_add(
                        out=v[:, :, :w], in0=v[:, :, :w], in1=v[:, :, w : 2 * w]
                    )
                    w //= 2
                feat = fo_pool.tile([P, tipb * C], mybir.dt.float16, tag="feat")
                fv = feat[:].rearrange("p (i w) -> p i w", w=C)
                nc.vector.tensor_add(
                    out=fv, in0=v[:, :, :C], in1=v[:, :, C : 2 * C]
                )
                # write the tile's means: contiguous P*tipb*C*2 bytes.
                nc.scalar.dma_start(
                    out=po[rows, :].rearrange("(p i) w -> p i w", p=P),
                    in_=fv,
                )
    nc.finalize()
    return nc


def shard_inputs(point_features):
    pf = np.asarray(point_features, dtype=np.float32).reshape(N_PILLARS, D)
    lo = np.ascontiguousarray(pf[:, :HD]).astype(np.float16)
    hi = np.ascontiguousarray(pf[:, HD:]).astype(np.float16)
    return [
        {"pf_lo": lo[r * NPC : (r + 1) * NPC], "pf_hi": hi[r * NPC : (r + 1) * NPC]}
        for r in range(N_CORES)
    ]


def assemble(results, voxel_coords):
    vc = np.asarray(voxel_coords)
    b = vc[:, 0].astype(np.int64)
    y = vc[:, 2].astype(np.int64)
    x = vc[:, 3].astype(np.int64)
    out = np.zeros((B, C, H, W), np.float32)
    inv_np = np.float32(1.0 / N_POINTS)
    for r in range(N_CORES):
        sl = slice(r * NPC, (r + 1) * NPC)
        feats = results[r]["po"].astype(np.float32) * inv_np
        out[b[sl], :, y[sl], x[sl]] = feats
    return out


def run(point_features, voxel_coords, trace=False, ipb=IPB, bufs=BUFS,
        **spmd_kwargs):
    in_maps = shard_inputs(point_features)
    nc = build_nc(ipb=ipb, bufs=bufs)
    br = run_bass_kernel_spmd(
        nc, in_maps, list(range(N_CORES)), trace=trace, **spmd_kwargs
    )
    return assemble(br.results, voxel_coords), br


def kernel(point_features, voxel_coords):
    out, _ = run(point_features, voxel_coords)
    return out
